# revision 5
# baseline (speedup 1.0000x reference)
"""AxialAttention TRN2 kernel.

Full-input contract: kernel(**inputs) takes the unsharded numpy inputs
(x [16,1280,1024], W_qkv [1024,3072], W_out [1024,1024], b_out [1024])
and returns the full output [16,1280,1024] fp32.

Sharding: data-parallel over batch, 2 images per NeuronCore x 8 cores.

Per-core plan (tokens n = 1280 = 256 ctx + 1024 img, heads=16, d=64):
  phase V : v = x @ W_v            -> DRAM scratch (natural [tok, 1024] layout)
  phase A : qT,kT = (x @ W_{q,k})^T  in SBUF  [feat, tok] (f32r)
  phase B : per (batch, head):
      S^T = kT.T @ qT   [key, query] layout (keys on partitions)
      expT = exp(S^T * scale)  (ACT, reads PSUM)  -- no max-subtraction
      img-img scores are block-diagonal (row-axial attention, 32-token
      rows); computed per 128-chunk and masked with a [128,128]
      block-diag 0/1 mask.
      AV: out^T = [V|1].T @ expT  -> PSUM [96, q]  (row 64 = sum(exp))
      normalize: zb = broadcast(row64); att = psum[0:64]/zb -> DRAM
  phase C : out = attn^T.T @ W_out + b_out
All matmuls run in float32r (~1e-4 relative accuracy, 4x faster than fp32).
"""

import numpy as np

import concourse.bass as bass
import concourse.tile as tile
from concourse import mybir
from concourse import bacc
import concourse.bass_utils as bass_utils

F32 = mybir.dt.float32
F32R = mybir.dt.float32r

B = 16                 # global batch
BPC = 2                # batches per core
NCORES = 8
CTX = 256
IMG = 32
N = CTX + IMG * IMG    # 1280 tokens
H = 1024               # hidden
HEADS = 16
D = 64
SCALE = D ** -0.5
P = 128
KT = H // P            # 8 hidden k-tiles
TOKT = N // P          # 10 token tiles
CHUNKS = [(0, 512), (512, 512), (1024, 256)]   # token chunks (<=512 fp32 moving)


def build_nc():
    nc = bacc.Bacc(None, target_bir_lowering=False)

    xT_d = nc.dram_tensor("xT", [BPC, H, N], F32R, kind="ExternalInput")
    wqkv_d = nc.dram_tensor("wqkv", [H, 3 * H], F32R, kind="ExternalInput")
    wout_d = nc.dram_tensor("wout", [H, H], F32R, kind="ExternalInput")
    bias_d = nc.dram_tensor("bias", [1, H], F32, kind="ExternalInput")
    out_d = nc.dram_tensor("out", [BPC, N, H], F32, kind="ExternalOutput")

    vtmp_d = nc.dram_tensor("vtmp", [BPC, N, H], F32R, kind="Internal")
    attnT_d = nc.dram_tensor("attnT", [BPC, H, N], F32R, kind="Internal")

    with tile.TileContext(nc) as tc:
        with (
            tc.tile_pool(name="persist", bufs=1) as persist,
            tc.tile_pool(name="xp", bufs=1) as xp,
            tc.tile_pool(name="qk", bufs=1) as qkp,
            tc.tile_pool(name="wqk", bufs=2) as wqkp,
            tc.tile_pool(name="wbig", bufs=1) as wbigp,
            tc.tile_pool(name="vh", bufs=2) as vhp,
            tc.tile_pool(name="ect", bufs=2) as ectp,
            tc.tile_pool(name="ei", bufs=2) as eip,
            tc.tile_pool(name="zz", bufs=2) as zzp,
            tc.tile_pool(name="att", bufs=2) as attp,
            tc.tile_pool(name="sbc", bufs=3) as sbcp,
            tc.tile_pool(name="cc", bufs=3) as ccp,
            tc.tile_pool(name="psA", bufs=2, space="PSUM") as psA,
            tc.tile_pool(name="psS", bufs=2, space="PSUM") as psS,
            tc.tile_pool(name="psV", bufs=3, space="PSUM") as psV,
        ):
            # --- constants ---
            mask = persist.tile([P, P], F32)            # img block-diag mask
            nc.gpsimd.memset(mask[:], 0.0)
            for a in range(4):
                nc.gpsimd.memset(mask[32 * a:32 * a + 32, 32 * a:32 * a + 32], 1.0)
            bias_sb = persist.tile([1, H], F32)
            nc.sync.dma_start(bias_sb[:], bias_d[:])
            bb = persist.tile([P, H], F32)               # bias broadcast
            nc.gpsimd.partition_broadcast(bb[:], bias_sb[0:1, :])

            wqkv_r = wqkv_d[:].rearrange("(kt p) f -> p kt f", p=P)   # [128, 8, 3072]
            wout_r = wout_d[:].rearrange("(kt p) f -> p kt f", p=P)   # [128, 8, 1024]

            for b in range(BPC):
                # ---------------- load xT ----------------
                xT = xp.tile([P, KT, N], F32R, tag="xT")
                nc.sync.dma_start(
                    xT[:], xT_d[b].rearrange("(kt p) t -> p kt t", p=P))

                # ---------------- phase V: v -> DRAM ----------------
                for ch in range(2):
                    wv = wbigp.tile([P, KT, 512], F32R, tag="wbig")
                    nc.sync.dma_start(wv[:], wqkv_r[:, :, 2 * H + 512 * ch:2 * H + 512 * (ch + 1)])
                    for t in range(TOKT):
                        ps = psA.tile([P, 512], F32, tag="psA")
                        for kt in range(KT):
                            nc.tensor.matmul(
                                ps[:], xT[:, kt, P * t:P * (t + 1)], wv[:, kt, :],
                                start=(kt == 0), stop=(kt == KT - 1))
                        vsb = sbcp.tile([P, 512], F32R, tag="vsb")
                        nc.any.tensor_copy(vsb[:], ps[:])
                        nc.sync.dma_start(
                            vtmp_d[b, P * t:P * (t + 1), 512 * ch:512 * (ch + 1)], vsb[:])

                # ------- heads in groups of 4 (A then B) -------
                for g in range(4):
                    # phase A: qT,kT for heads 4g..4g+3  (2 q M-tiles + 2 k M-tiles)
                    qkT = qkp.tile([P, 4, N], F32R, tag="qkT")
                    for m in range(4):
                        wcol = (256 * g + P * m) if m < 2 else (H + 256 * g + P * (m - 2))
                        wt = wqkp.tile([P, KT, P], F32R, tag="wqk")
                        nc.sync.dma_start(wt[:], wqkv_r[:, :, wcol:wcol + P])
                        for (c0, cw) in CHUNKS:
                            ps = psA.tile([P, 512], F32, tag="psA")
                            for kt in range(KT):
                                nc.tensor.matmul(
                                    ps[:, :cw], wt[:, kt, :], xT[:, kt, c0:c0 + cw],
                                    start=(kt == 0), stop=(kt == KT - 1))
                            nc.any.tensor_copy(qkT[:, m, c0:c0 + cw], ps[:, :cw])

                    # phase B: 4 heads
                    for hh in range(4):
                        h = 4 * g + hh
                        p0 = (hh % 2) * 64               # partition offset within M-tile
                        qi = hh // 2                     # q M-tile index (0..1)
                        ki = 2 + hh // 2                 # k M-tile index (2..3)

                        # V|ones tile for this head: [tok-part, tok-tile, 96]
                        vh = vhp.tile([P, TOKT, 96], F32R, tag="vh")
                        nc.scalar.activation(
                            vh[:, :, 64:96], xT[:, 0, :].rearrange("p (a b) -> p a b", b=32)[:, 0:TOKT, :],
                            mybir.ActivationFunctionType.Copy, scale=0.0, bias=1.0)
                        nc.sync.dma_start(
                            vh[:, :, 0:64],
                            vtmp_d[b].rearrange("(kt p) f -> p kt f", p=P)[:, :, 64 * h:64 * h + 64])

                        # S^T ctx: [256 keys, 1280 q] -> exp -> ect
                        ect = ectp.tile([P, 2, N], F32R, tag="ect")
                        for kc in range(2):
                            for (c0, cw) in CHUNKS:
                                ps = psS.tile([P, 512], F32, tag="psS")
                                nc.tensor.matmul(
                                    ps[:, :cw],
                                    qkT[p0:p0 + 64, ki, P * kc:P * (kc + 1)],
                                    qkT[p0:p0 + 64, qi, c0:c0 + cw],
                                    start=True, stop=True)
                                nc.scalar.activation(
                                    ect[:, kc, c0:c0 + cw], ps[:, :cw],
                                    mybir.ActivationFunctionType.Exp, scale=SCALE)

                        # S^T img diag chunks: 8 x [128,128], masked
                        eim = eip.tile([P, 8, P], F32R, tag="ei")
                        for c in range(8):
                            tok = CTX + P * c
                            ps = psS.tile([P, 512], F32, tag="psS")
                            nc.tensor.matmul(
                                ps[:, :P],
                                qkT[p0:p0 + 64, ki, tok:tok + P],
                                qkT[p0:p0 + 64, qi, tok:tok + P],
                                start=True, stop=True)
                            nc.scalar.activation(
                                eim[:, c, :], ps[:, :P],
                                mybir.ActivationFunctionType.Exp, scale=SCALE)
                            nc.vector.tensor_tensor(
                                eim[:, c, :], eim[:, c, :], mask[:], mybir.AluOpType.mult)

                        # AV + row sums
                        st = attp.tile([64, N], F32, tag="st")     # unnormalized out^T
                        zrow = zzp.tile([1, N], F32, tag="zrow")   # sum(exp) per q
                        for (c0, cw) in CHUNKS:
                            ps2 = psV.tile([P, 512], F32, tag="psV")
                            for kc in range(2):   # ctx keys
                                nc.tensor.matmul(
                                    ps2[:96, :cw], vh[:, kc, :], ect[:, kc, c0:c0 + cw],
                                    start=(kc == 0), stop=False)
                            nsub = cw // P
                            nimg = sum(1 for j in range(nsub) if c0 + P * j >= CTX)
                            seen = 0
                            for j in range(nsub):
                                qtok = c0 + P * j
                                if qtok < CTX:
                                    continue
                                seen += 1
                                nc.tensor.matmul(
                                    ps2[:96, P * j:P * (j + 1)],
                                    vh[:, qtok // P, :],
                                    eim[:, (qtok - CTX) // P, :],
                                    start=False, stop=(seen == nimg))
                            nc.vector.tensor_copy(st[:, c0:c0 + cw], ps2[0:64, :cw])
                            nc.vector.tensor_copy(zrow[0:1, c0:c0 + cw], ps2[64:65, :cw])

                        zb = zzp.tile([64, N], F32, tag="zb")
                        nc.gpsimd.partition_broadcast(zb[:], zrow[0:1, :])
                        nc.vector.reciprocal(zb[:], zb[:])
                        ath = attp.tile([64, N], F32R, tag="ath")
                        nc.vector.tensor_tensor(
                            ath[:], st[:], zb[:], mybir.AluOpType.mult)
                        nc.sync.dma_start(attnT_d[b, 64 * h:64 * h + 64, :], ath[:])

                # ---------------- phase C ----------------
                for ch in range(2):
                    wo = wbigp.tile([P, KT, 512], F32R, tag="wbig")
                    nc.sync.dma_start(wo[:], wout_r[:, :, 512 * ch:512 * (ch + 1)])
                    for t in range(TOKT):
                        aT = ccp.tile([P, KT, P], F32R, tag="aT")
                        nc.sync.dma_start(
                            aT[:],
                            attnT_d[b].rearrange("(kt p) t -> p kt t", p=P)[:, :, P * t:P * (t + 1)])
                        ps = psA.tile([P, 512], F32, tag="psA")
                        for kt in range(KT):
                            nc.tensor.matmul(
                                ps[:], aT[:, kt, :], wo[:, kt, :],
                                start=(kt == 0), stop=(kt == KT - 1))
                        osb = sbcp.tile([P, 512], F32, tag="osb")
                        nc.vector.tensor_tensor(
                            osb[:], ps[:], bb[:, 512 * ch:512 * (ch + 1)],
                            mybir.AluOpType.add)
                        nc.sync.dma_start(
                            out_d[b, P * t:P * (t + 1), 512 * ch:512 * (ch + 1)], osb[:])

    nc.finalize()
    return nc


_NC_CACHE = None


def _get_nc():
    global _NC_CACHE
    if _NC_CACHE is None:
        _NC_CACHE = build_nc()
    return _NC_CACHE


def run(x, W_qkv, W_out, b_out, trace=False):
    x = np.ascontiguousarray(np.asarray(x, dtype=np.float32))
    W_qkv = np.ascontiguousarray(np.asarray(W_qkv, dtype=np.float32))
    W_out = np.ascontiguousarray(np.asarray(W_out, dtype=np.float32))
    b_out = np.ascontiguousarray(np.asarray(b_out, dtype=np.float32)).reshape(1, H)

    nc = _get_nc()
    in_maps = []
    for c in range(NCORES):
        xc = x[BPC * c:BPC * (c + 1)]                      # [2, 1280, 1024]
        xT = np.ascontiguousarray(xc.transpose(0, 2, 1))   # [2, 1024, 1280]
        in_maps.append({"xT": xT, "wqkv": W_qkv, "wout": W_out, "bias": b_out})

    res = bass_utils.run_bass_kernel_spmd(
        nc, in_maps, core_ids=list(range(NCORES)), trace=trace)
    out = np.concatenate([r["out"] for r in res.results], axis=0)
    if trace:
        return out, res
    return out


def kernel(x, W_qkv, W_out, b_out):
    return run(x, W_qkv, W_out, b_out, trace=False)


# revision 6
# speedup vs baseline: 1.1610x; 1.1610x over previous
"""AxialAttention TRN2 kernel.

Full-input contract: kernel(**inputs) takes the unsharded numpy inputs
(x [16,1280,1024], W_qkv [1024,3072], W_out [1024,1024], b_out [1024])
and returns the full output [16,1280,1024] fp32.

Sharding: data-parallel over batch, 2 images per NeuronCore x 8 cores.

Per-core plan (tokens n = 1280 = 256 ctx + 1024 img, heads=16, d=64):
  phase V : v = x @ W_v            -> DRAM scratch (natural [tok, 1024] layout)
  phase A : qT,kT = (x @ W_{q,k})^T  in SBUF  [feat, tok] (f32r)
  phase B : per (batch, head):
      S^T = kT.T @ qT   [key, query] layout (keys on partitions)
      expT = exp(S^T * scale)  (ACT, reads PSUM)  -- no max-subtraction
      img-img scores are block-diagonal (row-axial attention, 32-token
      rows); computed per 128-chunk and masked with a [128,128]
      block-diag 0/1 mask.
      AV: out^T = [V|1].T @ expT  -> PSUM [96, q]  (row 64 = sum(exp))
      normalize: zb = broadcast(row64); att = psum[0:64]/zb -> DRAM
  phase C : out = attn^T.T @ W_out + b_out
All matmuls run in float32r (~1e-4 relative accuracy, 4x faster than fp32).
"""

import numpy as np

import concourse.bass as bass
import concourse.tile as tile
from concourse import mybir
from concourse import bacc
import concourse.bass_utils as bass_utils

F32 = mybir.dt.float32
F32R = mybir.dt.float32r

B = 16                 # global batch
BPC = 2                # batches per core
NCORES = 8
CTX = 256
IMG = 32
N = CTX + IMG * IMG    # 1280 tokens
H = 1024               # hidden
HEADS = 16
D = 64
SCALE = D ** -0.5
P = 128
KT = H // P            # 8 hidden k-tiles
TOKT = N // P          # 10 token tiles
CHUNKS = [(0, 512), (512, 512), (1024, 256)]   # token chunks (<=512 fp32 moving)


def build_nc():
    nc = bacc.Bacc(None, target_bir_lowering=False)

    xT_d = nc.dram_tensor("xT", [BPC, H, N], F32R, kind="ExternalInput")
    wqkv_d = nc.dram_tensor("wqkv", [H, 3 * H], F32R, kind="ExternalInput")
    wout_d = nc.dram_tensor("wout", [H, H], F32R, kind="ExternalInput")
    bias_d = nc.dram_tensor("bias", [1, H], F32, kind="ExternalInput")
    out_d = nc.dram_tensor("out", [BPC, N, H], F32, kind="ExternalOutput")

    vtmp_d = nc.dram_tensor("vtmp", [BPC, N, H], F32R, kind="Internal")
    attnT_d = nc.dram_tensor("attnT", [BPC, H, N], F32R, kind="Internal")

    with tile.TileContext(nc) as tc:
        with (
            tc.tile_pool(name="persist", bufs=1) as persist,
            tc.tile_pool(name="xp", bufs=1) as xp,
            tc.tile_pool(name="qk", bufs=1) as qkp,
            tc.tile_pool(name="wqk", bufs=2) as wqkp,
            tc.tile_pool(name="wbig", bufs=1) as wbigp,
            tc.tile_pool(name="vh", bufs=2) as vhp,
            tc.tile_pool(name="ect", bufs=2) as ectp,
            tc.tile_pool(name="ei", bufs=2) as eip,
            tc.tile_pool(name="zz", bufs=2) as zzp,
            tc.tile_pool(name="att", bufs=2) as attp,
            tc.tile_pool(name="sbc", bufs=3) as sbcp,
            tc.tile_pool(name="cc", bufs=3) as ccp,
            tc.tile_pool(name="psA", bufs=3, space="PSUM") as psA,
            tc.tile_pool(name="psS", bufs=2, space="PSUM") as psS,
            tc.tile_pool(name="psV", bufs=3, space="PSUM") as psV,
        ):
            # --- constants ---
            mask = persist.tile([P, P], F32)            # img block-diag mask
            nc.gpsimd.memset(mask[:], 0.0)
            for a in range(4):
                nc.gpsimd.memset(mask[32 * a:32 * a + 32, 32 * a:32 * a + 32], 1.0)
            bias_sb = persist.tile([1, H], F32)
            nc.sync.dma_start(bias_sb[:], bias_d[:])
            bb = persist.tile([P, H], F32)               # bias broadcast
            nc.gpsimd.partition_broadcast(bb[:], bias_sb[0:1, :])

            wqkv_r = wqkv_d[:].rearrange("(kt p) f -> p kt f", p=P)   # [128, 8, 3072]
            wout_r = wout_d[:].rearrange("(kt p) f -> p kt f", p=P)   # [128, 8, 1024]

            for b in range(BPC):
                # ---------------- load xT ----------------
                xT = xp.tile([P, KT, N], F32R, tag="xT")
                nc.sync.dma_start(
                    xT[:], xT_d[b].rearrange("(kt p) t -> p kt t", p=P))

                # ---------------- phase V: v -> DRAM ----------------
                for ch in range(2):
                    wv = wbigp.tile([P, KT, 512], F32R, tag="wbig")
                    nc.sync.dma_start(wv[:], wqkv_r[:, :, 2 * H + 512 * ch:2 * H + 512 * (ch + 1)])
                    for t in range(TOKT):
                        ps = psA.tile([P, 512], F32, tag="psA")
                        for kt in range(KT):
                            nc.tensor.matmul(
                                ps[:], xT[:, kt, P * t:P * (t + 1)], wv[:, kt, :],
                                start=(kt == 0), stop=(kt == KT - 1))
                        vsb = sbcp.tile([P, 512], F32R, tag="vsb")
                        nc.any.tensor_copy(vsb[:], ps[:])
                        nc.sync.dma_start(
                            vtmp_d[b, P * t:P * (t + 1), 512 * ch:512 * (ch + 1)], vsb[:])

                # ------- heads in groups of 4 (A then B) -------
                for g in range(4):
                    # phase A: qT,kT for heads 4g..4g+3  (2 q M-tiles + 2 k M-tiles)
                    qkT = qkp.tile([P, 4, N], F32R, tag="qkT")
                    for m in range(4):
                        wcol = (256 * g + P * m) if m < 2 else (H + 256 * g + P * (m - 2))
                        wt = wqkp.tile([P, KT, P], F32R, tag="wqk")
                        nc.sync.dma_start(wt[:], wqkv_r[:, :, wcol:wcol + P])
                        for (c0, cw) in CHUNKS:
                            ps = psA.tile([P, 512], F32, tag="psA")
                            for kt in range(KT):
                                nc.tensor.matmul(
                                    ps[:, :cw], wt[:, kt, :], xT[:, kt, c0:c0 + cw],
                                    start=(kt == 0), stop=(kt == KT - 1))
                            nc.any.tensor_copy(qkT[:, m, c0:c0 + cw], ps[:, :cw])

                    # phase B: 4 heads
                    for hh in range(4):
                        h = 4 * g + hh
                        p0 = (hh % 2) * 64               # partition offset within M-tile
                        qi = hh // 2                     # q M-tile index (0..1)
                        ki = 2 + hh // 2                 # k M-tile index (2..3)

                        # V|ones tile for this head: [tok-part, tok-tile, 96]
                        vh = vhp.tile([P, TOKT, 96], F32R, tag="vh")
                        nc.scalar.activation(
                            vh[:, :, 64:96], xT[:, 0, :].rearrange("p (a b) -> p a b", b=32)[:, 0:TOKT, :],
                            mybir.ActivationFunctionType.Copy, scale=0.0, bias=1.0)
                        nc.sync.dma_start(
                            vh[:, :, 0:64],
                            vtmp_d[b].rearrange("(kt p) f -> p kt f", p=P)[:, :, 64 * h:64 * h + 64])

                        # S^T ctx: [256 keys, 1280 q] -> exp -> ect
                        ect = ectp.tile([P, 2, N], F32R, tag="ect")
                        for kc in range(2):
                            for (c0, cw) in CHUNKS:
                                ps = psS.tile([P, 512], F32, tag="psS")
                                nc.tensor.matmul(
                                    ps[:, :cw],
                                    qkT[p0:p0 + 64, ki, P * kc:P * (kc + 1)],
                                    qkT[p0:p0 + 64, qi, c0:c0 + cw],
                                    start=True, stop=True)
                                nc.scalar.activation(
                                    ect[:, kc, c0:c0 + cw], ps[:, :cw],
                                    mybir.ActivationFunctionType.Exp, scale=SCALE)

                        # S^T img diag chunks: 8 x [128,128], masked
                        eim = eip.tile([P, 8, P], F32R, tag="ei")
                        for c in range(8):
                            tok = CTX + P * c
                            ps = psS.tile([P, 512], F32, tag="psS")
                            nc.tensor.matmul(
                                ps[:, :P],
                                qkT[p0:p0 + 64, ki, tok:tok + P],
                                qkT[p0:p0 + 64, qi, tok:tok + P],
                                start=True, stop=True)
                            nc.scalar.activation(
                                eim[:, c, :], ps[:, :P],
                                mybir.ActivationFunctionType.Exp, scale=SCALE)
                            nc.vector.tensor_tensor(
                                eim[:, c, :], eim[:, c, :], mask[:], mybir.AluOpType.mult)

                        # AV + row sums
                        st = attp.tile([64, N], F32, tag="st")     # unnormalized out^T
                        zrow = zzp.tile([1, N], F32, tag="zrow")   # sum(exp) per q
                        for (c0, cw) in CHUNKS:
                            ps2 = psV.tile([P, 512], F32, tag="psV")
                            for kc in range(2):   # ctx keys
                                nc.tensor.matmul(
                                    ps2[:96, :cw], vh[:, kc, :], ect[:, kc, c0:c0 + cw],
                                    start=(kc == 0), stop=False)
                            nsub = cw // P
                            nimg = sum(1 for j in range(nsub) if c0 + P * j >= CTX)
                            seen = 0
                            for j in range(nsub):
                                qtok = c0 + P * j
                                if qtok < CTX:
                                    continue
                                seen += 1
                                nc.tensor.matmul(
                                    ps2[:96, P * j:P * (j + 1)],
                                    vh[:, qtok // P, :],
                                    eim[:, (qtok - CTX) // P, :],
                                    start=False, stop=(seen == nimg))
                            nc.vector.tensor_copy(st[:, c0:c0 + cw], ps2[0:64, :cw])
                            nc.vector.tensor_copy(zrow[0:1, c0:c0 + cw], ps2[64:65, :cw])

                        zr = zzp.tile([1, N], F32, tag="zr")
                        nc.vector.reciprocal_approx_fast(out=zr[:], in_=zrow[:])
                        zb = zzp.tile([64, N], F32, tag="zb")
                        nc.gpsimd.partition_broadcast(zb[:], zr[0:1, :])
                        ath = attp.tile([64, N], F32R, tag="ath")
                        nc.vector.tensor_tensor(
                            ath[:], st[:], zb[:], mybir.AluOpType.mult)
                        nc.sync.dma_start(attnT_d[b, 64 * h:64 * h + 64, :], ath[:])

                # ---------------- phase C ----------------
                for ch in range(2):
                    wo = wbigp.tile([P, KT, 512], F32R, tag="wbig")
                    nc.sync.dma_start(wo[:], wout_r[:, :, 512 * ch:512 * (ch + 1)])
                    for t in range(TOKT):
                        aT = ccp.tile([P, KT, P], F32R, tag="aT")
                        nc.sync.dma_start(
                            aT[:],
                            attnT_d[b].rearrange("(kt p) t -> p kt t", p=P)[:, :, P * t:P * (t + 1)])
                        ps = psA.tile([P, 512], F32, tag="psA")
                        for kt in range(KT):
                            nc.tensor.matmul(
                                ps[:], aT[:, kt, :], wo[:, kt, :],
                                start=(kt == 0), stop=(kt == KT - 1))
                        osb = sbcp.tile([P, 512], F32, tag="osb")
                        nc.vector.tensor_tensor(
                            osb[:], ps[:], bb[:, 512 * ch:512 * (ch + 1)],
                            mybir.AluOpType.add)
                        nc.sync.dma_start(
                            out_d[b, P * t:P * (t + 1), 512 * ch:512 * (ch + 1)], osb[:])

    nc.finalize()
    return nc


_NC_CACHE = None


def _get_nc():
    global _NC_CACHE
    if _NC_CACHE is None:
        _NC_CACHE = build_nc()
    return _NC_CACHE


def run(x, W_qkv, W_out, b_out, trace=False):
    x = np.ascontiguousarray(np.asarray(x, dtype=np.float32))
    W_qkv = np.ascontiguousarray(np.asarray(W_qkv, dtype=np.float32))
    W_out = np.ascontiguousarray(np.asarray(W_out, dtype=np.float32))
    b_out = np.ascontiguousarray(np.asarray(b_out, dtype=np.float32)).reshape(1, H)

    nc = _get_nc()
    in_maps = []
    for c in range(NCORES):
        xc = x[BPC * c:BPC * (c + 1)]                      # [2, 1280, 1024]
        xT = np.ascontiguousarray(xc.transpose(0, 2, 1))   # [2, 1024, 1280]
        in_maps.append({"xT": xT, "wqkv": W_qkv, "wout": W_out, "bias": b_out})

    res = bass_utils.run_bass_kernel_spmd(
        nc, in_maps, core_ids=list(range(NCORES)), trace=trace)
    out = np.concatenate([r["out"] for r in res.results], axis=0)
    if trace:
        return out, res
    return out


def kernel(x, W_qkv, W_out, b_out):
    return run(x, W_qkv, W_out, b_out, trace=False)
